# revision 33
# baseline (speedup 1.0000x reference)
"""Trainium2 Bass kernel for per-batch (block-diagonal) attention.

Computes, for each batch b independently:
    q = x[b] @ Wq ; k = kv[b] @ Wk ; v = kv[b] @ Wv
    out[b] = softmax(q @ k^T) @ v

Sharding: data-parallel over B=8 across the 8 NeuronCores (one batch
element per core). Each core holds the full 64x64 weights.

Math used on-device (per core, x:[8192,64], kv:[1024,64]):
    k^T = Wk^T @ kv^T          (64x1024 fp16; lhsT=Wk needs no transpose)
    q^T = Wq^T @ x^T           (64x1024 fp16 per chunk, row-group packed)
    S^T = k q^T                -> scores^T tiles [128k, 1024q] (fp32 acc)
    P^T = exp(S^T)             (ACT, PSUM->SBUF, bf16 out)
    outT_aug = [v | 1]^T @ P^T (bf16, PSUM fp32 accumulate;
                                row 64 = softmax denominator)
    out = outT_aug[0:64].T / denom   (PE transpose back + DVE reciprocal*mul)

The steady state is ACT-bound (exp of 8.4M scores at 1 elem/cycle/lane,
1.2 GHz). The loop is software-pipelined so the ACT queue never gaps:
scores MMs double-buffer in PSUM, PV accumulates per key tile right
after its exp, the per-chunk epilogue (transpose-back + normalize +
store) is emitted two tiles into the next chunk, and x prefetch/cast/
transpose for chunk c+1 runs in the shadow of chunk c's exps.

dtype choices: fp16 for the scores matmul (scores accumulate in fp32
PSUM). bf16 for exp(S) (values up to e^~50 need fp32 range). Softmax
max-subtraction is skipped: scores ~ N(0, 64), |s|_max << 88, so exp()
cannot overflow fp32 and the result matches the reference softmax.
"""

from contextlib import ExitStack

import numpy as np

import concourse.mybir as mybir
from concourse import bacc
from concourse.masks import make_identity
from concourse.tile import TileContext

B, LQ, LK, NF = 8, 8192, 1024, 64
P = 128
CH = 512             # queries per PSUM-bank-sized slice
CP = 2 * CH          # 1024 queries per chunk
NC_CHUNKS = LQ // CP # 8
KT = LK // P         # 8 key tiles
VW = NF + 2          # v_aug width: v | ones | zero-pad (even)
F32 = mybir.dt.float32
F16 = mybir.dt.float16
BF16 = mybir.dt.bfloat16
EXP = mybir.ActivationFunctionType.Exp

_CACHE: dict = {}


def _build_nc():
    nc = bacc.Bacc("TRN2", target_bir_lowering=False, debug=False)
    x = nc.dram_tensor("x", [LQ, NF], F32, kind="ExternalInput").ap()
    kv = nc.dram_tensor("kv", [LK, NF], F32, kind="ExternalInput").ap()
    wq = nc.dram_tensor("Wq", [NF, NF], F32, kind="ExternalInput").ap()
    wk = nc.dram_tensor("Wk", [NF, NF], F32, kind="ExternalInput").ap()
    wv = nc.dram_tensor("Wv", [NF, NF], F32, kind="ExternalInput").ap()
    y = nc.dram_tensor("y", [LQ, NF], F32, kind="ExternalOutput").ap()

    with TileContext(nc) as tc, ExitStack() as ctx:
        singles = ctx.enter_context(tc.tile_pool(name="singles", bufs=1))

        # ---- input DMAs first so they overlap identity/transpose setup ----
        # query mapping q = c*1024 + 8p + 2*jp + par -> 2KB contiguous DMA
        # lines per partition (vs 256B with the naive (q,f) partition split)
        # preload the exp table set ASAP so the ~2.7us load overlaps prologue
        warm = singles.tile([P, 1], F32)
        nc.vector.memset(warm, 0.0)
        nc.scalar.activation(out=warm, in_=warm, func=EXP)

        xin = ctx.enter_context(tc.tile_pool(name="xin", bufs=4))
        x_view = x.rearrange("(c p jp par) f -> c p jp par f", p=P, jp=4, par=2)
        x_tiles: dict = {}

        def emit_x_dma(c, nsplit=1):
            x_sb = xin.tile([P, 4, 2, NF], F32, tag="x", name=f"x_sb{c}")
            x_tiles[c] = x_sb
            step = 4 // nsplit
            for s in range(nsplit):
                sl = slice(s * step, (s + 1) * step)
                nc.sync.dma_start(out=x_sb[:, sl, :, :], in_=x_view[c, :, sl, :, :])

        emit_x_dma(0)

        # key mapping k = 8p + t (tile t holds keys = t mod 8; a pure
        # permutation -- v_aug rows follow kvT so softmax is unaffected)
        kv_sb = singles.tile([P, KT, NF], F32)
        kv_v = kv.rearrange("(p t) f -> p t f", p=P)
        nc.scalar.dma_start(out=kv_sb, in_=kv_v)
        wq_sb = singles.tile([NF, NF], F32)
        wk_sb = singles.tile([NF, NF], F32)
        wv_sb = singles.tile([NF, NF], F32)
        nc.scalar.dma_start(out=wq_sb, in_=wq)
        nc.scalar.dma_start(out=wk_sb, in_=wk)
        nc.scalar.dma_start(out=wv_sb, in_=wv)
        emit_x_dma(1)

        ident = singles.tile([P, P], F32)
        make_identity(nc, ident)
        ident16 = singles.tile([P, P], F16)
        nc.gpsimd.tensor_copy(ident16, ident)
        identb = singles.tile([P, P], BF16)
        nc.gpsimd.tensor_copy(identb, ident)

        # v_aug [128 keys, KT, 66] = [v | 1 | 0] (filled late in prologue)
        v_aug = singles.tile([P, KT, VW], BF16)
        wv16 = singles.tile([NF, NF], F16)
        wk16 = singles.tile([NF, NF], F16)
        # Wq (fp16), duplicated into partitions 64:128 for the packed
        # row-group-B qT MM
        wq16d = singles.tile([P, NF], F16)
        kv16 = singles.tile([P, KT, NF], F16)
        kvT = singles.tile([NF, LK], F16)
        # k^T = Wk^T @ kv^T [64, 1024] fp16, duplicated into partitions
        # 64:128 for the packed row-group-B score MMs
        kT = singles.tile([P, LK], F16)

        # ---- pools for the main loop ----
        x16_pool = ctx.enter_context(tc.tile_pool(name="x16", bufs=2))
        xT_pool = ctx.enter_context(tc.tile_pool(name="xT", bufs=2))
        qT_pool = ctx.enter_context(tc.tile_pool(name="qT", bufs=2))
        pT_pool = ctx.enter_context(tc.tile_pool(name="pT", bufs=6))
        pvT_pool = ctx.enter_context(tc.tile_pool(name="pvT", bufs=2))
        out_pool = ctx.enter_context(tc.tile_pool(name="outsb", bufs=2))
        rec_pool = ctx.enter_context(tc.tile_pool(name="rec", bufs=4))

        # PSUM budget (8 banks): scores 2x2 + xt 1 + ot 1 + pv 2 = 8
        sc_ps_pool = ctx.enter_context(
            tc.tile_pool(name="sc_ps", bufs=2, space="PSUM")
        )
        xt_ps_pool = ctx.enter_context(
            tc.tile_pool(name="xt_ps", bufs=1, space="PSUM")
        )
        ot_ps_pool = ctx.enter_context(
            tc.tile_pool(name="ot_ps", bufs=1, space="PSUM")
        )
        pv_ps_pool = ctx.enter_context(
            tc.tile_pool(name="pv_ps", bufs=1, space="PSUM")
        )

        xT_tiles: dict = {}

        def emit_xt(c):
            if c in xT_tiles:
                return
            # cast to fp16 (gpsimd; DVE for chunk 0 latency), then stacked PE
            # transpose: xT partitions 0:64 = even subtiles' features,
            # 64:128 = odd subtiles' features
            x_sb = x_tiles.pop(c)
            x16 = x16_pool.tile([P, 4, 2, NF], F16, tag="x16", name=f"x16_{c}")
            if c == 0:
                nc.vector.tensor_copy(x16, x_sb)
            else:
                nc.gpsimd.tensor_copy(x16, x_sb)
            xt_ps = xt_ps_pool.tile([P, 4, P], F16, tag="xt", name=f"xt_ps{c}")
            for i in range(4):
                nc.tensor.transpose(xt_ps[:, i, :], x16[:, i, :, :], ident16)
            xTc = xT_pool.tile([P, 4, P], F16, tag="xT", name=f"xT{c}")
            nc.vector.tensor_copy(xTc, xt_ps)
            xT_tiles[c] = xTc

        emit_xt(0)

        pT_tiles: dict = {}

        qT_tiles: dict = {}

        def emit_qt(c):
            if c in qT_tiles:
                return
            # q^T = Wq^T x^T for this chunk: row-group packing puts even
            # subtiles' q^T on partitions 0:64 and odd on 64:128 (the odd
            # MM lands there directly via col-group 64)
            xTc = xT_tiles[c]
            q_ps = xt_ps_pool.tile([P, CH], F32, tag="xt", name=f"q_ps{c}")
            nc.tensor.matmul(
                q_ps[:NF, :], lhsT=wq16d[:NF, :], rhs=xTc[:NF],
                start=True, stop=True, tile_position=(0, 0),
            )
            nc.tensor.matmul(
                q_ps[NF:, :], lhsT=wq16d[NF:, :], rhs=xTc[NF:],
                start=True, stop=True, tile_position=(64, 64),
            )
            qTc = qT_pool.tile([P, CH], F16, tag="qT", name=f"qT{c}")
            nc.vector.tensor_copy(qTc, q_ps)
            qT_tiles[c] = qTc

        def emit_scores_exp(c, t):
            # scores^T tile: 2 row-group-packed MMs (even/odd q subtiles)
            qTc = qT_tiles[c]
            s_ps = sc_ps_pool.tile([P, CP], F32, tag="sc", name=f"s_ps{c}_{t}")
            nc.tensor.matmul(
                s_ps[:, :CH],
                lhsT=kT[:NF, t * P : (t + 1) * P],
                rhs=qTc[:NF],
                start=True, stop=True,
                tile_position=(0, 0),
            )
            nc.tensor.matmul(
                s_ps[:, CH:],
                lhsT=kT[NF:, t * P : (t + 1) * P],
                rhs=qTc[NF:],
                start=True, stop=True,
                tile_position=(64, 0),
            )
            pT_t = pT_pool.tile([P, CP], BF16, tag="pT", name=f"pT{c}_{t}")
            nc.scalar.activation(out=pT_t, in_=s_ps, func=EXP)
            pT_tiles[(c, t)] = pT_t

        # ---- prologue: kv^T, W^T, A^T, U^T, v_aug; the first score tiles
        # are emitted as soon as the U^T half they need exists, so ACT
        # starts exp-ing while the rest of the prologue still runs ----
        if True:
            def kvt_pair(i):
                j = 2 * i
                kt_ps = ot_ps_pool.tile([P, P], F16, tag="ot", name="kt_ps")
                nc.tensor.transpose(kt_ps, kv16[:, j : j + 2, :], ident16)
                nc.vector.tensor_copy(kvT[:, j * P : (j + 1) * P], kt_ps[:NF, :])
                nc.vector.tensor_copy(
                    kvT[:, (j + 1) * P : (j + 2) * P], kt_ps[NF:, :]
                )

            def kt_half(h):
                kt_mm_ps = sc_ps_pool.tile([NF, CH], F32, tag="sc",
                                           name="kt_mm_ps")
                nc.tensor.matmul(
                    kt_mm_ps, lhsT=wk16, rhs=kvT[:, h * CH : (h + 1) * CH],
                    start=True, stop=True,
                )
                nc.vector.tensor_copy(kT[:NF, h * CH : (h + 1) * CH], kt_mm_ps)
                nc.vector.tensor_copy(
                    kT[NF:, h * CH : (h + 1) * CH],
                    kT[:NF, h * CH : (h + 1) * CH],
                )

            # weight casts (off the critical path; no W transposes needed --
            # wk/wq enter their MMs as the stationary [d, e] operand)
            nc.gpsimd.tensor_copy(wk16, wk_sb)
            nc.gpsimd.tensor_copy(wq16d[:NF, :], wq_sb)
            nc.vector.tensor_copy(wq16d[NF:, :], wq16d[:NF, :])
            nc.gpsimd.tensor_copy(wv16, wv_sb)
            # kv half 0 -> kvT cols 0:512
            nc.vector.tensor_copy(kv16[:, : KT // 2, :], kv_sb[:, : KT // 2, :])
            kvt_pair(0)
            kvt_pair(1)
            kt_half(0)
            emit_qt(0)

            # score tiles 0-3 only need k^T cols 0:512 -- start ACT now;
            # the kv half-1 -> k^T half-1 chain then hides under their exps
            for t in range(4):
                emit_scores_exp(0, t)

            nc.vector.tensor_copy(kv16[:, KT // 2 :, :], kv_sb[:, KT // 2 :, :])
            kvt_pair(2)
            kvt_pair(3)
            kt_half(1)

            for t in range(4, KT):
                emit_scores_exp(0, t)

            # v_aug fill: 8 MMs into one PSUM bank, one strided drain
            v_ps = pv_ps_pool.tile([P, KT, NF], F32, tag="pv", name="v_ps")
            for tt in range(KT):
                nc.tensor.matmul(
                    v_ps[:, tt, :], lhsT=kvT[:, tt * P : (tt + 1) * P],
                    rhs=wv16, start=True, stop=True,
                )
            nc.vector.tensor_copy(v_aug[:, :, :NF], v_ps)
            nc.vector.memset(v_aug[:, :, NF : NF + 1], 1.0)
            nc.vector.memset(v_aug[:, :, NF + 1 :], 0.0)

        # pre-emit chunk 1's x-transpose, q^T and first two score tiles so
        # the first chunk boundary's (cold, slow) PV backlog on the PE queue
        # cannot starve ACT of its next exp input
        emit_xt(1)
        emit_qt(1)
        emit_scores_exp(1, 0)
        emit_scores_exp(1, 1)

        pv_tiles: dict = {}
        pvT_tiles: dict = {}

        def emit_pvT_drain(c, last=False):
            # PSUM -> SBUF drain of the accumulated [v|1]^T P^T (frees pv bank
            # for the next chunk; emitted before PV(c+1, t0)). In the tail
            # ACT is idle, so it drains the second half in parallel.
            pv_ps = pv_tiles.pop(c)
            pvT = pvT_pool.tile([NF + 1, CP], BF16, tag="pvT", name=f"pvT{c}")
            nc.vector.tensor_copy(pvT[:, :CH], pv_ps[: NF + 1, :CH])
            if last:
                nc.scalar.copy(pvT[:, CH:], pv_ps[: NF + 1, CH:])
            else:
                nc.vector.tensor_copy(pvT[:, CH:], pv_ps[: NF + 1, CH:])
            pvT_tiles[c] = pvT

        y_view = y.rearrange("(c p j) f -> c p j f", p=P, j=8)

        out_tiles: dict = {}

        def emit_epilogue_r(c, r, last=False):
            # transpose back to [128 q, 65], normalize; store after r=1.
            # pvT col j*128+p (j = 4r+s) is query 8p + 2s + r = 8p + jj.
            pvT = pvT_tiles[c]
            if r == 0:
                out_tiles[c] = out_pool.tile([P, 8, NF], F32, tag="osb",
                                             name=f"out_sb{c}")
            out_sb = out_tiles[c]
            if last:
                # the scores ring is free after the final exp -- use two of
                # its banks so r=0/r=1 epilogues run in parallel
                ot_ps = sc_ps_pool.tile([P, 4, NF + 2], BF16, tag="sc",
                                        name=f"ot_ps{c}_{r}")
            else:
                ot_ps = ot_ps_pool.tile([P, 4, NF + 2], BF16, tag="ot",
                                        name=f"ot_ps{c}_{r}")
            rec = rec_pool.tile([P, 4], F32, tag="rec", name=f"rec{c}_{r}")
            for s in range(4):
                j = 4 * r + s
                nc.tensor.transpose(
                    ot_ps[:, s, : NF + 1],
                    pvT[:, j * P : (j + 1) * P],
                    identb[: NF + 1, : NF + 1],
                )
            nc.vector.reciprocal(rec, ot_ps[:, :, NF])
            for s in range(4):
                if last and s >= 2:
                    nc.scalar.mul(
                        out_sb[:, 2 * s + r, :], ot_ps[:, s, :NF],
                        rec[:, s : s + 1],
                    )
                else:
                    nc.vector.tensor_scalar_mul(
                        out_sb[:, 2 * s + r, :], ot_ps[:, s, :NF],
                        rec[:, s : s + 1],
                    )
            if r == 1:
                pvT_tiles.pop(c)
                out_tiles.pop(c)
                nc.sync.dma_start(out=y_view[c], in_=out_sb)

        def emit_pv(g):
            # PV for key tile g (accumulates over t; row 64 = denom)
            c, t = divmod(g, KT)
            if t == 0:
                pv_tiles[c] = pv_ps_pool.tile([P, CP], F32, tag="pv",
                                              name=f"pv_ps{c}")
            pv_ps = pv_tiles[c]
            pT_t = pT_tiles.pop((c, t))
            for half in range(2):
                nc.tensor.matmul(
                    pv_ps[:VW, half * CH : (half + 1) * CH],
                    lhsT=v_aug[:, t, :],
                    rhs=pT_t[:, half * CH : (half + 1) * CH],
                    start=(t == 0), stop=(t == KT - 1),
                )

        # ---- main loop over global key-tile index. Scores/exp run one
        # tile ahead and PV lags two tiles, so at a chunk boundary the PE
        # queue reaches the next chunk's first scores MMs ~0.4us sooner
        # (ACT stays dense); the lagged PV work fills the PE's slack ----
        PVLAG = 2
        for g in range(NC_CHUNKS * KT):
            c, t = divmod(g, KT)
            if t == 2 and c > 0:
                emit_pvT_drain(c - 1)

            if (c, t) not in pT_tiles:
                emit_scores_exp(c, t)
            if g + 1 < NC_CHUNKS * KT:
                cn, tn = divmod(g + 1, KT)
                if (cn, tn) not in pT_tiles:
                    emit_scores_exp(cn, tn)
            if g >= PVLAG:
                emit_pv(g - PVLAG)

            if t == 0 and c + 2 < NC_CHUNKS:
                emit_x_dma(c + 2)
            if t == 3 and c > 0:
                emit_epilogue_r(c - 1, 0)
            if t == 5 and c > 0:
                emit_epilogue_r(c - 1, 1)
            if t == 4 and c + 1 < NC_CHUNKS:
                emit_xt(c + 1)
            if t == 5 and c + 1 < NC_CHUNKS:
                emit_qt(c + 1)
            if t == KT - 1:
                xT_tiles.pop(c)
                qT_tiles.pop(c)

        for g in range(NC_CHUNKS * KT - PVLAG, NC_CHUNKS * KT):
            emit_pv(g)
        emit_pvT_drain(NC_CHUNKS - 1, last=True)
        emit_epilogue_r(NC_CHUNKS - 1, 0, last=True)
        emit_epilogue_r(NC_CHUNKS - 1, 1, last=True)

    nc.compile()
    return nc


def get_nc():
    if "nc" not in _CACHE:
        _CACHE["nc"] = _build_nc()
    return _CACHE["nc"]


def run(inputs: dict, trace: bool = False):
    """Run on the 8 NeuronCores. Returns (out [8,8192,64], exec_time_ns)."""
    from concourse.bass_utils import run_bass_kernel_spmd

    nc = get_nc()
    in_maps = [
        {
            "x": np.ascontiguousarray(inputs["x"][b]),
            "kv": np.ascontiguousarray(inputs["kv"][b]),
            "Wq": np.asarray(inputs["Wq"]),
            "Wk": np.asarray(inputs["Wk"]),
            "Wv": np.asarray(inputs["Wv"]),
        }
        for b in range(B)
    ]
    res = run_bass_kernel_spmd(
        nc, in_maps, core_ids=list(range(B)), trace=trace
    )
    out = np.stack([res.results[b]["y"] for b in range(B)])
    return out, res.exec_time_ns


def kernel(**inputs) -> np.ndarray:
    out, _ = run(inputs, trace=False)
    return out
